# revision 2
# baseline (speedup 1.0000x reference)
"""DeepSeekV2-style MLA prefill attention on 8 Trainium2 NeuronCores.

Tensor-parallel over heads: each core owns 2 of the 16 q heads (q_nope only —
the rope half of q is discarded by the reference), replicates the single
latent kv head, computes its heads' causal attention and a partial o-proj;
the host sums the 8 partial outputs.

All matmuls run in bf16 (fp32 PSUM accumulation). hs^T and all weights are
exchanged pre-tiled so every DMA has a contiguous source block.

Layout strategy (matmuls contract over the partition dim):
  - projections computed transposed: q^T/k^T/v^T [d, s] = W^T.T @ hs^T
  - scores^T [l, q] = k^T_chunk.T @ q^T_tile (fp32 PSUM), diagonal tiles
    narrowed to their causal width
  - softmax without max-subtraction (scores are provably small: |s| < ~6)
  - exp on ScalarE (PSUM -> SBUF bf16); only the [128,128] diagonal block
    needs a triangular mask multiply
  - PV flipped vs v1: stationary = v chunk [l,128], moving = exp tile
    [l, up-to-512 q] -> out[d, q] accumulates in PSUM across l-chunks.
    This streams 512 cols per weight load (vs 129) and yields out already
    in the [d, q] layout the o-proj wants (no per-block PE transposes).
  - softmax denominator: VectorE accumulates exp tiles into an fp16 acc
    (4x DVE mode), GpSimd partition_all_reduce -> den[*, q] broadcast,
    reciprocal + normalize fused into the PSUM->SBUF evacuation multiply.
  - o-proj emitted per 512-col s-chunk, matmul pairs woven between
    attention l-chunk steps to fill PE bubbles left by the scalar exp.
"""

import numpy as np
import ml_dtypes
from contextlib import ExitStack

import concourse.bass as bass
import concourse.mybir as mybir
import concourse.tile as tile
from concourse import bacc
from concourse.bass_isa import ReduceOp
from concourse.bass_utils import run_bass_kernel_spmd
from concourse.masks import make_identity

B, S, HID = 2, 2048, 2048
H, D_NOPE, D_ROPE = 16, 128, 64
D = D_NOPE
N_CORES = 8
HPC = H // N_CORES          # heads per core
BS = B * S
SCALE = float(D_NOPE) ** -0.5

F32 = mybir.dt.float32
F16 = mybir.dt.float16
BF16 = mybir.dt.bfloat16

HC = HID // 128             # 16 hid chunks
ST2 = BS // 1024            # 4 wide s tiles
LCB = S // 128              # 16 l chunks per batch
NQC = BS // 512             # 8 output s-chunks

_cache = {}


def _build():
    if "nc" in _cache:
        return _cache["nc"]

    nc = bacc.Bacc("TRN2", target_bir_lowering=False, debug=False,
                   num_devices=N_CORES)
    # hsT tiled: [st2, hid_chunk, 128, 1024] contiguous blocks
    hsT_d = nc.dram_tensor("hsTt", [ST2, HC, 128, 1024], BF16,
                           kind="ExternalInput").ap()
    # weights pre-tiled on host so every chunk DMA has a contiguous source
    wqT_d = nc.dram_tensor("wqTt", [HC, 128, HPC * D], BF16,
                           kind="ExternalInput").ap()
    wkT_d = nc.dram_tensor("wkTt", [HC, 128, D], BF16,
                           kind="ExternalInput").ap()
    wvT_d = nc.dram_tensor("wvTt", [HC, 128, D], BF16,
                           kind="ExternalInput").ap()
    woT_d = nc.dram_tensor("woTt", [128, HPC, HC, 128], BF16,
                           kind="ExternalInput").ap()
    # output tiled: [hid_chunk, s_chunk, 128, 512]
    outT_d = nc.dram_tensor("outTt", [HC, NQC, 128, 512], BF16,
                            kind="ExternalOutput").ap()

    with ExitStack() as ctx:
        tc = ctx.enter_context(tile.TileContext(nc))
        persist = ctx.enter_context(tc.tile_pool(name="persist", bufs=1))

        wq_sb = persist.tile([128, HC, HPC * D], BF16, tag="wq_sb")
        wk_sb = persist.tile([128, HC, D], BF16, tag="wk_sb")
        wv_sb = persist.tile([128, HC, D], BF16, tag="wv_sb")
        wo_sb = persist.tile([128, HPC, HC, 128], BF16, tag="wo_sb")
        qT_sb = persist.tile([128, HPC, BS], BF16, tag="qT_sb")
        kT_sb = persist.tile([128, BS], BF16, tag="kT_sb")
        vT_sb = persist.tile([128, BS], BF16, tag="vT_sb")
        v_sb = persist.tile([128, B * LCB, D], BF16, tag="v_sb")
        tri_f = persist.tile([128, 128], F32, tag="tri_f")
        tri_sb = persist.tile([128, 128], BF16, tag="tri_sb")
        ident_b = persist.tile([128, 128], BF16, tag="ident_b")
        outT_sb = persist.tile([128, HPC, BS], BF16, tag="outT_sb")

        # ---- constants ----
        make_identity(nc, ident_b[:])
        # tri[x, y] = 1 where x <= y else 0 (diag-block causal mask)
        nc.gpsimd.memset(tri_f[:], 1.0)
        nc.gpsimd.affine_select(
            out=tri_f[:], in_=tri_f[:],
            compare_op=mybir.AluOpType.is_ge,
            fill=0.0, base=0,
            pattern=[[1, 128]], channel_multiplier=-1)
        nc.vector.tensor_copy(tri_sb[:], tri_f[:])

        # ---- phase 1: projections q^T (per head), k^T, v^T ----
        with tc.tile_pool(name="ps_proj", bufs=1, space="PSUM") as ps_proj, \
             tc.tile_pool(name="hs_pool", bufs=16) as hs_pool:
            for st2 in range(ST2):
                # 8 PSUM banks: (2 heads + k + v) x 2 halves
                pq = [[ps_proj.tile([128, 512], F32, tag=f"pq{h}_{hf}",
                                    name=f"pq{h}_{hf}")
                       for hf in range(2)] for h in range(HPC)]
                pk = [ps_proj.tile([128, 512], F32, tag=f"pk_{hf}",
                                   name=f"pk_{hf}") for hf in range(2)]
                pv = [ps_proj.tile([128, 512], F32, tag=f"pv_{hf}",
                                   name=f"pv_{hf}") for hf in range(2)]
                for hcx in range(HC):
                    hst = hs_pool.tile([128, 1024], BF16, tag="hst")
                    nc.sync.dma_start(hst[:], hsT_d[st2, hcx])
                    if st2 == 0:
                        nc.sync.dma_start(wq_sb[:, hcx, :], wqT_d[hcx])
                        nc.sync.dma_start(wk_sb[:, hcx, :], wkT_d[hcx])
                        nc.sync.dma_start(wv_sb[:, hcx, :], wvT_d[hcx])
                        if hcx == HC - 1:
                            nc.sync.dma_start(wo_sb[:], woT_d)
                    first, last = hcx == 0, hcx == HC - 1
                    for hf in range(2):
                        hr = hst[:, hf * 512:(hf + 1) * 512]
                        for h in range(HPC):
                            nc.tensor.matmul(
                                pq[h][hf][:],
                                wq_sb[:, hcx, h * D:(h + 1) * D],
                                hr, start=first, stop=last)
                        nc.tensor.matmul(pk[hf][:], wk_sb[:, hcx, :], hr,
                                         start=first, stop=last)
                        nc.tensor.matmul(pv[hf][:], wv_sb[:, hcx, :], hr,
                                         start=first, stop=last)
                for hf in range(2):
                    sl = slice(st2 * 1024 + hf * 512,
                               st2 * 1024 + (hf + 1) * 512)
                    for h in range(HPC):
                        nc.scalar.copy(qT_sb[:, h, sl], pq[h][hf][:])
                    nc.scalar.copy(kT_sb[:, sl], pk[hf][:])
                    nc.vector.tensor_copy(vT_sb[:, sl], pv[hf][:])

        # ---- phases 1b/2/3 share one 8-bank PSUM pool:
        #      tp(1) + ps(3) + outp(2) + po(2) = 8
        main_ps = ctx.enter_context(
            tc.tile_pool(name="main_ps", bufs=1, space="PSUM"))
        att_sb = ctx.enter_context(tc.tile_pool(name="att_sb", bufs=2))
        norm_sb = ctx.enter_context(tc.tile_pool(name="norm_sb", bufs=2))
        stage = ctx.enter_context(tc.tile_pool(name="stage", bufs=4))

        # phase 1b: v^T -> v (natural [l, d] layout) via PE transpose
        for lc in range(B * LCB):
            tp = main_ps.tile([128, 128], BF16, tag="tp", bufs=1, name="tp")
            nc.tensor.transpose(
                tp[:], vT_sb[:, lc * 128:(lc + 1) * 128], ident_b[:])
            nc.vector.tensor_copy(v_sb[:, lc, :], tp[:])

        # ---- o-proj steps, woven between attention l-chunk iterations ----
        pending = []
        cp_state = [0]

        def oproj_step(gq, hcx):
            sl = slice(gq * 512, (gq + 1) * 512)
            po = main_ps.tile([128, 512], F32, tag="po", bufs=2, name="po")
            nc.tensor.matmul(po[:], wo_sb[:, 0, hcx, :], outT_sb[:, 0, sl],
                             start=True, stop=False)
            nc.tensor.matmul(po[:], wo_sb[:, 1, hcx, :], outT_sb[:, 1, sl],
                             start=False, stop=True)
            ob = stage.tile([128, 512], BF16, tag="ob", name="ob")
            if cp_state[0] % 2 == 0:
                nc.vector.tensor_copy(ob[:], po[:])
            else:
                nc.scalar.copy(ob[:], po[:])
            cp_state[0] += 1
            nc.sync.dma_start(outT_d[hcx, gq], ob[:])

        def drain(n):
            for _ in range(n):
                if not pending:
                    return
                gq, hcx = pending.pop(0)
                oproj_step(gq, hcx)

        # ---- phase 2: causal attention per (batch, q-tile, head) ----
        for b in range(B):
            qoff = b * S
            for qt in range(S // 512):
                Q = qt * 512
                nl = Q // 128 + 4
                gq = b * (S // 512) + qt
                for h in range(HPC):
                    outp = main_ps.tile([128, 512], F32, tag="outp", bufs=2,
                                        name="outp")
                    exs = att_sb.tile([128, LCB, 512], BF16, tag="exs",
                                      name="exs")
                    acc = norm_sb.tile([128, 512], F16, tag="acc", name="acc")

                    def score(lc):
                        m = lc - Q // 128
                        w0 = max(m, 0) * 128
                        ps = main_ps.tile([128, 512], F32, tag="ps", bufs=3,
                                          name="ps")
                        nc.tensor.matmul(
                            ps[:, w0:512],
                            kT_sb[:, qoff + lc * 128: qoff + (lc + 1) * 128],
                            qT_sb[:, h, qoff + Q + w0: qoff + Q + 512],
                            start=True, stop=True)
                        return ps

                    pss = {0: score(0)}
                    if nl > 1:
                        pss[1] = score(1)
                    for lc in range(nl):
                        m = lc - Q // 128
                        w0 = max(m, 0) * 128
                        ps = pss.pop(lc)
                        exv = exs[:, lc, :]
                        nc.scalar.activation(
                            exv[:, w0:512], ps[:, w0:512],
                            mybir.ActivationFunctionType.Exp, scale=SCALE)
                        if m >= 0:
                            nc.vector.tensor_mul(
                                exv[:, w0:w0 + 128], exv[:, w0:w0 + 128],
                                tri_sb[:])
                        # denominator accumulation (fp16, 4x DVE mode)
                        if lc == 0:
                            nc.vector.tensor_copy(acc[:], exv[:])
                        else:
                            nc.vector.tensor_add(
                                acc[:, w0:512], acc[:, w0:512],
                                exv[:, w0:512])
                        # PV: out[d, q] += v_lc.T @ ex_lc, stop flag set on
                        # the diagonal block whose columns finish here
                        vlc = v_sb[:, b * LCB + lc, :]
                        st = lc == 0
                        if m >= 0:
                            nc.tensor.matmul(
                                outp[:, w0:w0 + 128], vlc,
                                exv[:, w0:w0 + 128], start=st, stop=True)
                            if w0 + 128 < 512:
                                nc.tensor.matmul(
                                    outp[:, w0 + 128:512], vlc,
                                    exv[:, w0 + 128:512], start=st,
                                    stop=False)
                        else:
                            nc.tensor.matmul(outp[:], vlc, exv[:],
                                             start=st, stop=False)
                        if lc + 2 < nl:
                            pss[lc + 2] = score(lc + 2)
                        drain(1)
                    # normalize: den = colsum(acc), outT = outp / den
                    den = norm_sb.tile([128, 512], F32, tag="den", name="den")
                    nc.gpsimd.partition_all_reduce(den[:], acc[:], 128,
                                                   ReduceOp.add)
                    rcp = norm_sb.tile([128, 512], F32, tag="rcp", name="rcp")
                    nc.vector.reciprocal(rcp[:], den[:])
                    nc.vector.tensor_mul(
                        outT_sb[:, h, qoff + Q: qoff + Q + 512],
                        outp[:], rcp[:])
                for hcx in range(HC):
                    pending.append((gq, hcx))
        drain(len(pending))

    nc.compile()
    _cache["nc"] = nc
    return nc


def _bf(x):
    return np.ascontiguousarray(x).astype(ml_dtypes.bfloat16)


def _in_maps(inputs):
    hs = np.asarray(inputs["hidden_states"], np.float32).reshape(BS, HID).T
    hsT = _bf(hs)                                   # [HID, BS]
    # tile into contiguous [st2, hc, 128, 1024] blocks
    hsTt = np.ascontiguousarray(
        hsT.reshape(HC, 128, ST2, 1024).transpose(2, 0, 1, 3))
    Wq = np.asarray(inputs["Wq"], np.float32)
    Wkv = np.asarray(inputs["Wkv"], np.float32)
    Wo = np.asarray(inputs["Wo"], np.float32)

    wkT = _bf(Wkv[:D, :].T).reshape(HC, 128, D)
    wvT = _bf(Wkv[D:2 * D, :].T).reshape(HC, 128, D)
    Wq_r = Wq.reshape(H, D_NOPE + D_ROPE, HID)

    in_maps = []
    for c in range(N_CORES):
        heads = range(c * HPC, (c + 1) * HPC)
        wqT = _bf(np.concatenate(
            [Wq_r[h, :D_NOPE, :] for h in heads], 0).T).reshape(
                HC, 128, HPC * D)
        woT = _bf(np.concatenate(
            [Wo[:, h * D:(h + 1) * D].T for h in heads], 0))
        woTt = np.ascontiguousarray(
            woT.reshape(HPC, 128, HC, 128).transpose(1, 0, 2, 3))
        in_maps.append({
            "hsTt": hsTt, "wqTt": wqT, "wkTt": wkT, "wvTt": wvT,
            "woTt": woTt,
        })
    return in_maps


def _gather(results):
    acc = results[0]["outTt"].astype(np.float32)
    for r in results[1:]:
        acc = acc + r["outTt"].astype(np.float32)
    # [hc, qc, 128, 512] -> outT [HID, BS] -> [B, S, HID]
    outT = acc.transpose(0, 2, 1, 3).reshape(HID, BS)
    return np.ascontiguousarray(outT.T).reshape(B, S, HID)


def run(inputs, trace=False, **kw):
    nc = _build()
    res = run_bass_kernel_spmd(nc, _in_maps(inputs), list(range(N_CORES)),
                               trace=trace, **kw)
    return _gather(res.results), res


def kernel(**inputs):
    out, _ = run(inputs)
    return out


# revision 6
# speedup vs baseline: 1.1594x; 1.1594x over previous
"""DeepSeekV2-style MLA prefill attention on 8 Trainium2 NeuronCores.

Tensor-parallel over heads: each core owns 2 of the 16 q heads (q_nope only —
the rope half of q is discarded by the reference), replicates the single
latent kv head, computes its heads' causal attention and a partial o-proj;
the host sums the 8 partial outputs.

All matmuls run in bf16 (fp32 PSUM accumulation). hs^T and all weights are
exchanged pre-tiled so every DMA has a contiguous source block.

Layout strategy (matmuls contract over the partition dim):
  - projections computed transposed: q^T/k^T/v^T [d, s] = W^T.T @ hs^T
  - scores^T [l, q] = k^T_chunk.T @ q^T_tile (fp32 PSUM), diagonal tiles
    narrowed to their causal width
  - softmax without max-subtraction (scores are provably small: |s| < ~6)
  - exp on ScalarE (PSUM -> SBUF bf16); only the [128,128] diagonal block
    needs a triangular mask multiply
  - PV flipped vs v1: stationary = v chunk [l,128], moving = exp tile
    [l, up-to-512 q] -> out[d, q] accumulates in PSUM across l-chunks.
    This streams 512 cols per weight load (vs 129) and yields out already
    in the [d, q] layout the o-proj wants (no per-block PE transposes).
  - softmax denominator: VectorE accumulates exp tiles into an fp16 acc
    (4x DVE mode), GpSimd partition_all_reduce -> den[*, q] broadcast,
    reciprocal + normalize fused into the PSUM->SBUF evacuation multiply.
  - o-proj emitted per 512-col s-chunk, matmul pairs woven between
    attention l-chunk steps to fill PE bubbles left by the scalar exp.
"""

import numpy as np
import ml_dtypes
from contextlib import ExitStack

import concourse.bass as bass
import concourse.mybir as mybir
import concourse.tile as tile
from concourse import bacc
from concourse.bass_isa import ReduceOp
from concourse.bass_utils import run_bass_kernel_spmd
from concourse.masks import make_identity

B, S, HID = 2, 2048, 2048
H, D_NOPE, D_ROPE = 16, 128, 64
D = D_NOPE
N_CORES = 8
HPC = H // N_CORES          # heads per core
BS = B * S
SCALE = float(D_NOPE) ** -0.5

F32 = mybir.dt.float32
F16 = mybir.dt.float16
BF16 = mybir.dt.bfloat16

HC = HID // 128             # 16 hid chunks
ST2 = BS // 1024            # 4 wide s tiles
LCB = S // 128              # 16 l chunks per batch
NQC = BS // 512             # 8 output s-chunks

_cache = {}


def _build():
    if "nc" in _cache:
        return _cache["nc"]

    nc = bacc.Bacc("TRN2", target_bir_lowering=False, debug=False,
                   num_devices=N_CORES)
    # hsT tiled: [st2, hid_chunk, 128, 1024] contiguous blocks
    hsT_d = nc.dram_tensor("hsTt", [ST2, HC, 128, 1024], BF16,
                           kind="ExternalInput").ap()
    # weights pre-tiled on host so every chunk DMA has a contiguous source
    wqT_d = nc.dram_tensor("wqTt", [HC, 128, HPC * D], BF16,
                           kind="ExternalInput").ap()
    wkT_d = nc.dram_tensor("wkTt", [HC, 128, D], BF16,
                           kind="ExternalInput").ap()
    wvT_d = nc.dram_tensor("wvTt", [HC, 128, D], BF16,
                           kind="ExternalInput").ap()
    woT_d = nc.dram_tensor("woTt", [128, HPC, HC, 128], BF16,
                           kind="ExternalInput").ap()
    # output tiled: [hid_chunk, s_chunk, 128, 512]
    outT_d = nc.dram_tensor("outTt", [HC, NQC, 128, 512], BF16,
                            kind="ExternalOutput").ap()

    with ExitStack() as ctx:
        tc = ctx.enter_context(tile.TileContext(nc))
        persist = ctx.enter_context(tc.tile_pool(name="persist", bufs=1))

        wq_sb = persist.tile([128, HC, HPC * D], BF16, tag="wq_sb")
        wk_sb = persist.tile([128, HC, D], BF16, tag="wk_sb")
        wv_sb = persist.tile([128, HC, D], BF16, tag="wv_sb")
        wo_sb = persist.tile([128, HPC, HC, 128], BF16, tag="wo_sb")
        qT_sb = persist.tile([128, HPC, BS], BF16, tag="qT_sb")
        kT_sb = persist.tile([128, BS], BF16, tag="kT_sb")
        vT_sb = persist.tile([128, BS], BF16, tag="vT_sb")
        v_sb = persist.tile([128, B * LCB, D], BF16, tag="v_sb")
        tri_f = persist.tile([128, 128], F32, tag="tri_f")
        tri_sb = persist.tile([128, 128], BF16, tag="tri_sb")
        ident_b = persist.tile([128, 128], BF16, tag="ident_b")
        outT_sb = persist.tile([128, HPC, BS], BF16, tag="outT_sb")

        # ---- constants ----
        make_identity(nc, ident_b[:])
        # tri[x, y] = 1 where x <= y else 0 (diag-block causal mask)
        nc.gpsimd.memset(tri_f[:], 1.0)
        nc.gpsimd.affine_select(
            out=tri_f[:], in_=tri_f[:],
            compare_op=mybir.AluOpType.is_ge,
            fill=0.0, base=0,
            pattern=[[1, 128]], channel_multiplier=-1)
        nc.vector.tensor_copy(tri_sb[:], tri_f[:])

        # ---- phase 1: projections q^T (per head), k^T, v^T ----
        with tc.tile_pool(name="ps_proj", bufs=1, space="PSUM") as ps_proj, \
             tc.tile_pool(name="hs_pool", bufs=8) as hs_pool:
            for st2 in range(ST2):
                # 8 PSUM banks: (2 heads + k + v) x 2 halves
                pq = [[ps_proj.tile([128, 512], F32, tag=f"pq{h}_{hf}",
                                    name=f"pq{h}_{hf}")
                       for hf in range(2)] for h in range(HPC)]
                pk = [ps_proj.tile([128, 512], F32, tag=f"pk_{hf}",
                                   name=f"pk_{hf}") for hf in range(2)]
                pv = [ps_proj.tile([128, 512], F32, tag=f"pv_{hf}",
                                   name=f"pv_{hf}") for hf in range(2)]
                for hcx in range(HC):
                    hst = hs_pool.tile([128, 1024], BF16, tag="hst")
                    nc.sync.dma_start(hst[:], hsT_d[st2, hcx])
                    if st2 == 0:
                        # weight chunk DMAs issue from the gpsimd queue
                        # so they don't serialize behind hs issues
                        nc.gpsimd.dma_start(wq_sb[:, hcx, :], wqT_d[hcx])
                        nc.gpsimd.dma_start(wk_sb[:, hcx, :], wkT_d[hcx])
                        nc.gpsimd.dma_start(wv_sb[:, hcx, :], wvT_d[hcx])
                        if hcx == HC - 1:
                            nc.gpsimd.dma_start(wo_sb[:], woT_d)
                    first, last = hcx == 0, hcx == HC - 1
                    for hf in range(2):
                        hr = hst[:, hf * 512:(hf + 1) * 512]
                        for h in range(HPC):
                            nc.tensor.matmul(
                                pq[h][hf][:],
                                wq_sb[:, hcx, h * D:(h + 1) * D],
                                hr, start=first, stop=last)
                        nc.tensor.matmul(pk[hf][:], wk_sb[:, hcx, :], hr,
                                         start=first, stop=last)
                        nc.tensor.matmul(pv[hf][:], wv_sb[:, hcx, :], hr,
                                         start=first, stop=last)
                for hf in range(2):
                    sl = slice(st2 * 1024 + hf * 512,
                               st2 * 1024 + (hf + 1) * 512)
                    for h in range(HPC):
                        nc.scalar.copy(qT_sb[:, h, sl], pq[h][hf][:])
                    nc.scalar.copy(kT_sb[:, sl], pk[hf][:])
                    nc.vector.tensor_copy(vT_sb[:, sl], pv[hf][:])

        # ---- phases 1b/2/3 share one 8-bank PSUM pool:
        #      tp(1) + ps(3) + outp(2) + po(2) = 8
        main_ps = ctx.enter_context(
            tc.tile_pool(name="main_ps", bufs=1, space="PSUM"))
        att_sb = ctx.enter_context(tc.tile_pool(name="att_sb", bufs=2))
        norm_sb = ctx.enter_context(tc.tile_pool(name="norm_sb", bufs=2))
        stage = ctx.enter_context(tc.tile_pool(name="stage", bufs=4))

        # phase 1b: v^T -> v (natural [l, d] layout) via PE transpose
        for lc in range(B * LCB):
            tp = main_ps.tile([128, 128], BF16, tag="tp", bufs=1, name="tp")
            nc.tensor.transpose(
                tp[:], vT_sb[:, lc * 128:(lc + 1) * 128], ident_b[:])
            nc.vector.tensor_copy(v_sb[:, lc, :], tp[:])

        # ---- o-proj steps, woven between attention l-chunk iterations ----
        pending = []
        cp_state = [0]

        def oproj_step(gq, hcx):
            sl = slice(gq * 512, (gq + 1) * 512)
            po = main_ps.tile([128, 512], F32, tag="po", bufs=2, name="po")
            nc.tensor.matmul(po[:], wo_sb[:, 0, hcx, :], outT_sb[:, 0, sl],
                             start=True, stop=False)
            nc.tensor.matmul(po[:], wo_sb[:, 1, hcx, :], outT_sb[:, 1, sl],
                             start=False, stop=True)
            ob = stage.tile([128, 512], BF16, tag="ob", name="ob")
            if cp_state[0] % 2 == 0:
                nc.vector.tensor_copy(ob[:], po[:])
            else:
                nc.scalar.copy(ob[:], po[:])
            cp_state[0] += 1
            nc.sync.dma_start(outT_d[hcx, gq], ob[:])

        def drain(n):
            # keep one full q-tile of lag so drained steps never wait on the
            # in-flight normalize chain (which would stall the in-order PE)
            for _ in range(n):
                if len(pending) <= HC:
                    return
                gq, hcx = pending.pop(0)
                oproj_step(gq, hcx)

        def drain_all():
            while pending:
                gq, hcx = pending.pop(0)
                oproj_step(gq, hcx)

        # ---- phase 2: causal attention per (batch, q-tile, head) ----
        for b in range(B):
            qoff = b * S
            for qt in range(S // 512):
                Q = qt * 512
                nl = Q // 128 + 4
                gq = b * (S // 512) + qt
                for h in range(HPC):
                    outp = main_ps.tile([128, 512], F32, tag="outp", bufs=2,
                                        name="outp")
                    exs = att_sb.tile([128, LCB, 512], BF16, tag="exs",
                                      name="exs")
                    acc = norm_sb.tile([128, 512], F16, tag="acc", name="acc")

                    def score(lc):
                        m = lc - Q // 128
                        w0 = max(m, 0) * 128
                        ps = main_ps.tile([128, 512], F32, tag="ps", bufs=3,
                                          name="ps")
                        nc.tensor.matmul(
                            ps[:, w0:512],
                            kT_sb[:, qoff + lc * 128: qoff + (lc + 1) * 128],
                            qT_sb[:, h, qoff + Q + w0: qoff + Q + 512],
                            start=True, stop=True)
                        return ps

                    pss = {0: score(0)}
                    if nl > 1:
                        pss[1] = score(1)
                    for lc in range(nl):
                        m = lc - Q // 128
                        w0 = max(m, 0) * 128
                        ps = pss.pop(lc)
                        exv = exs[:, lc, :]
                        nc.scalar.activation(
                            exv[:, w0:512], ps[:, w0:512],
                            mybir.ActivationFunctionType.Exp, scale=SCALE)
                        if m >= 0:
                            nc.vector.tensor_mul(
                                exv[:, w0:w0 + 128], exv[:, w0:w0 + 128],
                                tri_sb[:])
                        # denominator accumulation (fp16, 4x DVE mode)
                        if lc == 0:
                            nc.vector.tensor_copy(acc[:], exv[:])
                        else:
                            nc.vector.tensor_add(
                                acc[:, w0:512], acc[:, w0:512],
                                exv[:, w0:512])
                        # PV: out[d, q] += v_lc.T @ ex_lc, stop flag set on
                        # the diagonal block whose columns finish here
                        vlc = v_sb[:, b * LCB + lc, :]
                        st = lc == 0
                        if m >= 0:
                            nc.tensor.matmul(
                                outp[:, w0:w0 + 128], vlc,
                                exv[:, w0:w0 + 128], start=st, stop=True)
                            if w0 + 128 < 512:
                                nc.tensor.matmul(
                                    outp[:, w0 + 128:512], vlc,
                                    exv[:, w0 + 128:512], start=st,
                                    stop=False)
                        else:
                            nc.tensor.matmul(outp[:], vlc, exv[:],
                                             start=st, stop=False)
                        if lc + 2 < nl:
                            pss[lc + 2] = score(lc + 2)
                        drain(1)
                    # normalize: den = colsum(acc), outT = outp / den
                    den = norm_sb.tile([128, 512], F32, tag="den", name="den")
                    nc.gpsimd.partition_all_reduce(den[:], acc[:], 128,
                                                   ReduceOp.add)
                    rcp = norm_sb.tile([128, 512], F32, tag="rcp", name="rcp")
                    nc.vector.reciprocal_approx_fast(rcp[:], den[:])
                    nc.vector.tensor_mul(
                        outT_sb[:, h, qoff + Q: qoff + Q + 512],
                        outp[:], rcp[:])
                for hcx in range(HC):
                    pending.append((gq, hcx))
        drain_all()

    nc.compile()
    _cache["nc"] = nc
    return nc


def _bf(x):
    return np.ascontiguousarray(x).astype(ml_dtypes.bfloat16)


def _in_maps(inputs):
    hs = np.asarray(inputs["hidden_states"], np.float32).reshape(BS, HID).T
    hsT = _bf(hs)                                   # [HID, BS]
    # tile into contiguous [st2, hc, 128, 1024] blocks
    hsTt = np.ascontiguousarray(
        hsT.reshape(HC, 128, ST2, 1024).transpose(2, 0, 1, 3))
    Wq = np.asarray(inputs["Wq"], np.float32)
    Wkv = np.asarray(inputs["Wkv"], np.float32)
    Wo = np.asarray(inputs["Wo"], np.float32)

    wkT = _bf(Wkv[:D, :].T).reshape(HC, 128, D)
    wvT = _bf(Wkv[D:2 * D, :].T).reshape(HC, 128, D)
    Wq_r = Wq.reshape(H, D_NOPE + D_ROPE, HID)

    in_maps = []
    for c in range(N_CORES):
        heads = range(c * HPC, (c + 1) * HPC)
        wqT = _bf(np.concatenate(
            [Wq_r[h, :D_NOPE, :] for h in heads], 0).T).reshape(
                HC, 128, HPC * D)
        woT = _bf(np.concatenate(
            [Wo[:, h * D:(h + 1) * D].T for h in heads], 0))
        woTt = np.ascontiguousarray(
            woT.reshape(HPC, 128, HC, 128).transpose(1, 0, 2, 3))
        in_maps.append({
            "hsTt": hsTt, "wqTt": wqT, "wkTt": wkT, "wvTt": wvT,
            "woTt": woTt,
        })
    return in_maps


def _gather(results):
    acc = results[0]["outTt"].astype(np.float32)
    for r in results[1:]:
        acc = acc + r["outTt"].astype(np.float32)
    # [hc, qc, 128, 512] -> outT [HID, BS] -> [B, S, HID]
    outT = acc.transpose(0, 2, 1, 3).reshape(HID, BS)
    return np.ascontiguousarray(outT.T).reshape(B, S, HID)


def run(inputs, trace=False, **kw):
    nc = _build()
    res = run_bass_kernel_spmd(nc, _in_maps(inputs), list(range(N_CORES)),
                               trace=trace, **kw)
    return _gather(res.results), res


def kernel(**inputs):
    out, _ = run(inputs)
    return out
